# revision 20
# baseline (speedup 1.0000x reference)
"""Trainium2 Bass kernel for the temporal-encoder problem.

Computes, for inputs event_time [8, 2048] (event_type unused by the math):
  scores    [8, 2048, 2048] = exp(-d^2/ls^2) * (1 + tanh((|d|/200 - l)/s)) * tril
  embedding [8, 2048, 512]  = sin/cos positional encoding of event_time
  t_diff    [8, 2048, 2048] = t[:, :, None] - t[:, None, :]

Sharding: data-parallel over batch B=8 across the 8 NeuronCores (core b
handles batch b). All heavy compute and all output bytes are produced on
device; the host only computes a handful of scalar constants and stacks
the per-core results.
"""

import math
from contextlib import ExitStack

import numpy as np

import concourse.bacc as bacc
import concourse.bass as bass
import concourse.mybir as mybir
import concourse.tile as tile
from concourse.bass_utils import run_bass_kernel_spmd

D_MODEL = 512
T_MAX = 200.0
B = 8
L = 2048
P = 128
NBLK = L // P  # 16
F32 = mybir.dt.float32
AF = mybir.ActivationFunctionType
OP = mybir.AluOpType
MAGIC = float(np.float32(1.5 * 2**23))  # add/sub rounds f32 to nearest int
TWO_PI = 2.0 * math.pi

# layout of the two packed constant inputs
BC_W = L + 2 * D_MODEL  # [1, 3072] broadcast source: t | inv_pos/2pi | phase/2pi
CC_W = NBLK + 1 + P  # [128, 145] per-partition: t_cols | tanh_bias | tri


def build_nc(inv_ls: float, tanh_scale: float) -> bass.Bass:
    # Bacc (not raw Bass): its compile() runs generate_event_semaphores,
    # which splits sync-waits that exceed the per-instruction ISA limits.
    nc = bacc.Bacc()

    bcsrc = nc.declare_dram_parameter("bcsrc", [1, BC_W], F32, isOutput=False)
    colsrc = nc.declare_dram_parameter("colsrc", [P, CC_W], F32, isOutput=False)
    scores = nc.declare_dram_parameter("scores", [L, L], F32, isOutput=True)
    tdiff = nc.declare_dram_parameter("tdiff", [L, L], F32, isOutput=True)
    emb = nc.declare_dram_parameter("emb", [L, D_MODEL], F32, isOutput=True)

    with tile.TileContext(nc) as tc, ExitStack() as ctx:
        const = ctx.enter_context(tc.tile_pool(name="const", bufs=1))
        embp = ctx.enter_context(tc.tile_pool(name="embp", bufs=3))
        work = ctx.enter_context(tc.tile_pool(name="work", bufs=3))

        # ---- constants: exactly two input DMAs -------------------------
        bigbc = const.tile([P, BC_W], F32)  # row consts bcast along partitions
        nc.gpsimd.dma_start(out=bigbc[:], in_=bcsrc[:].to_broadcast([P, BC_W]))
        cc = const.tile([P, CC_W], F32)
        nc.sync.dma_start(out=cc[:], in_=colsrc[:])

        T_row = bigbc[:, 0:L]
        invrow = bigbc[:, L : L + D_MODEL]
        phrow = bigbc[:, L + D_MODEL : L + 2 * D_MODEL]
        t_cols = cc[:, 0:NBLK]
        tanh_bias_ap = cc[:, NBLK : NBLK + 1]
        tri_ap = cc[:, NBLK + 1 : CC_W]

        zeros_sb = const.tile([P, L], F32)
        nc.vector.memset(zeros_sb[:], 0.0)

        # Primers: absorb the const-DMA completion sems into each engine's
        # vector clock with 1-element ops, so real instructions (esp. the
        # TensorScalar struct with few sync-wait slots) never need to wait
        # on the input DMAs themselves.
        pr1 = const.tile([1, 1], F32)
        pr2 = const.tile([1, 1], F32)
        pr3 = const.tile([1, 1], F32)
        nc.vector.tensor_copy(pr1[:], bigbc[0:1, 0:1])
        nc.vector.tensor_copy(pr2[:], cc[0:1, 0:1])
        nc.scalar.copy(pr3[:], cc[0:1, 0:1])

        # ---- embedding phase (grouped first: keeps Sin's ACT table
        # load separate from the exp/tanh set used by the scores phase) --
        for k in range(NBLK):
            i0 = k * P
            u_sb = embp.tile([P, D_MODEL], F32, tag="u")
            # u = te/2pi = t_col * inv_pos/2pi + phase/2pi
            nc.vector.scalar_tensor_tensor(
                u_sb[:], invrow, t_cols[:, k : k + 1], phrow,
                op0=OP.mult, op1=OP.add,
            )
            r_sb = embp.tile([P, D_MODEL], F32, tag="r")
            # r = round(u) via the f32 magic-number trick; any integer works
            # since sin(2pi(u - r)) == sin(2pi u).
            nc.vector.tensor_scalar(
                r_sb[:], u_sb[:], MAGIC, MAGIC, op0=OP.add, op1=OP.subtract
            )
            # frac = u - r, computed in place over r as (r * -1) + u: the
            # scalar_tensor_tensor ISA struct has a single sync-wait slot,
            # so it must only ever depend on same-engine producers.
            nc.vector.scalar_tensor_tensor(
                r_sb[:], r_sb[:], -1.0, u_sb[:], op0=OP.mult, op1=OP.add
            )
            emb_sb = embp.tile([P, D_MODEL], F32, tag="emb")
            # 1-element absorber: takes the DMA slot-release wait on DVE so
            # the Sin activation stays within its sync-wait budget.
            nc.vector.tensor_copy(emb_sb[0:1, 0:1], zeros_sb[0:1, 0:1])
            nc.scalar.activation(emb_sb[:], r_sb[:], AF.Sin, scale=TWO_PI)
            nc.sync.dma_start(out=emb[i0 : i0 + P, :], in_=emb_sb[:])

        # ---- scores + t_diff phase ------------------------------------
        for k in range(NBLK):
            i0 = k * P
            W = P * (k + 1)  # lower-triangular width incl. diagonal block
            td = work.tile([P, L], F32, tag="td")
            # td = t_col - T_row = t[i] - t[j]
            nc.vector.tensor_scalar(
                td[:], T_row, t_cols[:, k : k + 1], -1.0,
                op0=OP.subtract, op1=OP.mult,
            )
            nc.sync.dma_start(out=tdiff[i0 : i0 + P, :], in_=td[:])

            dabs = work.tile([P, L], F32, tag="dabs")
            # |td| = max(-td, td)
            nc.vector.scalar_tensor_tensor(
                dabs[:, :W], td[:, :W], -1.0, td[:, :W], op0=OP.mult, op1=OP.max
            )
            d2 = work.tile([P, L], F32, tag="d2")
            nc.scalar.activation(d2[:, :W], td[:, :W], AF.Square, scale=inv_ls)
            kern = work.tile([P, L], F32, tag="kern")
            nc.scalar.activation(kern[:, :W], d2[:, :W], AF.Exp, scale=-1.0)
            # tanh((|d|/T - l)/s) in place
            nc.scalar.activation(
                dabs[:, :W], dabs[:, :W], AF.Tanh, scale=tanh_scale,
                bias=tanh_bias_ap,
            )
            # scores = (tanh_u + 1) * kern, written in place over kern so the
            # DMA slot-release wait lands on ACT Exp (which has spare slots)
            # rather than on this single-wait-slot STT instruction.
            nc.vector.scalar_tensor_tensor(
                kern[:, :W], dabs[:, :W], 1.0, kern[:, :W], op0=OP.add, op1=OP.mult
            )
            # mask the diagonal 128x128 block
            nc.vector.scalar_tensor_tensor(
                kern[:, i0 : i0 + P], kern[:, i0 : i0 + P], 0.0, tri_ap,
                op0=OP.bypass, op1=OP.mult,
            )
            nc.sync.dma_start(out=scores[i0 : i0 + P, 0:W], in_=kern[:, :W])
            if W < L:
                nc.sync.dma_start(
                    out=scores[i0 : i0 + P, W:L], in_=zeros_sb[:, : L - W]
                )
    nc.compile()
    return nc


def make_in_maps(event_time: np.ndarray, tanh_bias: float) -> list[dict]:
    i = np.arange(D_MODEL, dtype=np.float64)
    pos_vec = np.power(10000.0, 2.0 * i / D_MODEL)
    inv_pos = 1.0 / pos_vec
    phase = np.where(i % 2 == 0, 0.0, math.pi / 2.0)
    inv2pi = (inv_pos / TWO_PI).astype(np.float32)
    ph2pi = (phase / TWO_PI).astype(np.float32)
    tri = np.tril(np.ones((P, P), dtype=np.float32))

    in_maps = []
    for b in range(B):
        t = np.ascontiguousarray(event_time[b].astype(np.float32))  # [2048]
        bc = np.concatenate([t, inv2pi, ph2pi])[None, :]  # [1, 3072]
        ccsrc = np.concatenate(
            [
                np.ascontiguousarray(t.reshape(NBLK, P).T),  # [128, 16]
                np.full((P, 1), tanh_bias, dtype=np.float32),
                tri,
            ],
            axis=1,
        )  # [128, 145]
        in_maps.append(
            {"bcsrc": bc, "colsrc": np.ascontiguousarray(ccsrc.astype(np.float32))}
        )
    return in_maps


_cached = {}


def _get_nc(inv_ls: float, tanh_scale: float) -> bass.Bass:
    key = (inv_ls, tanh_scale)
    if key not in _cached:
        _cached[key] = build_nc(inv_ls, tanh_scale)
    return _cached[key]


def kernel(event_type, event_time, length_scale_param, gate_params):
    event_time = np.asarray(event_time, dtype=np.float32)
    lsp = float(np.asarray(length_scale_param))
    gp = np.asarray(gate_params, dtype=np.float64)

    ls = math.log1p(math.exp(lsp))  # softplus
    l = 1.0 / (1.0 + math.exp(-gp[0]))  # sigmoid
    s = 1.0 / (1.0 + math.exp(-gp[1]))
    inv_ls = 1.0 / ls
    tanh_scale = 1.0 / (T_MAX * s)
    tanh_bias = -l / s

    nc = _get_nc(inv_ls, tanh_scale)
    in_maps = make_in_maps(event_time, tanh_bias)
    res = run_bass_kernel_spmd(nc, in_maps, core_ids=list(range(B)))

    scores = np.stack([res.results[b]["scores"] for b in range(B)])
    emb = np.stack([res.results[b]["emb"] for b in range(B)])
    tdiff = np.stack([res.results[b]["tdiff"] for b in range(B)])
    return scores, emb, tdiff
